# revision 13
# baseline (speedup 1.0000x reference)
"""AdaClusteringAttention Trainium2 kernel (8 NeuronCores, batch/head parallel).

Algorithm (per batch*head row b, cluster row = clusters[b % 8]):
  q_c/k_c/v_c = per-cluster means (segment-sum * 1/count)      [C=513, D=128]
  qk = q_c @ k_c^T ; a = softmax(qk) * counts ; a /= rowsum    [C, C]
  v  = a @ v_c ; out[n] = v[cluster[n]] ; a0 = a[:, 0]

Device strategy per core (8 rows each, all sharing ONE cluster row):
  - host: stable-sort tokens by cluster; pad each 128-cluster block's token
    list to a multiple of 128 (padding uniform across cores => one SPMD graph)
  - dma_gather token rows (512B each) in sorted order
  - segment sums become per-128-token-chunk matmuls against tiny one-hot
    blocks (tokens of a chunk all fall inside one 128-cluster block)
  - cluster-level attention: qkT = k_c q_c^T, aT = exp(qkT + ln(count[e]))
    (count-weighted softmax w/o max-subtraction; scale cancels in renorm)
  - v_out = aT^T @ [v_c | 1] gives both numerator and rowsum s
  - out tokens via one-hot-transpose matmuls in sorted order, then
    dma_scatter_add back to token order (outputs are zero-initialized;
    pad rows go to a dump row NSEQ)
"""

import os
import sys

import numpy as np

B0, H, NSEQ, D = 8, 8, 4096, 128
B = B0 * H
C = 513
NBLK = 5          # ceil(C/128) cluster blocks
CPAD = NBLK * 128  # 640
NCORES = 8
ROWS = B // NCORES  # 8 rows per core
LN_NEG = -88.0      # exp(-88) == 0 in f32/bf16

_f32 = np.float32


def _bf16():
    import ml_dtypes
    return ml_dtypes.bfloat16


# ----------------------------------------------------------------- host meta

class Meta:
    pass


def build_meta(clusters: np.ndarray) -> Meta:
    """clusters [B0, NSEQ] int32 -> per-core sorted/padded index structures.

    The chunk->block map (graph structure) is shared across cores; only the
    tensor *values* (indices, one-hot blocks, counts) differ per core.
    """
    m = Meta()
    assert clusters.shape == (B0, NSEQ)
    counts = np.zeros((B0, CPAD), np.int64)
    for i in range(B0):
        counts[i, :C] = np.bincount(clusters[i], minlength=C)
    blk_tok = counts.reshape(B0, NBLK, 128).sum(-1)          # [B0, NBLK]
    T_m = np.maximum(128, (np.ceil(blk_tok.max(0) / 128) * 128).astype(np.int64))
    m.T_m = tuple(int(x) for x in T_m)
    m.TT = int(T_m.sum())
    m.Tc = m.TT // 128
    offs = np.concatenate([[0], np.cumsum(T_m)]).astype(np.int64)
    m.offs = offs
    blk_of_chunk = []
    for mm in range(NBLK):
        blk_of_chunk += [mm] * (m.T_m[mm] // 128)
    m.blk_of_chunk = tuple(blk_of_chunk)
    first = {}
    last = {}
    for t, mm in enumerate(m.blk_of_chunk):
        first.setdefault(mm, t)
        last[mm] = t
    m.first_chunk = first
    m.last_chunk = last

    bf16 = _bf16()
    m.cores = []
    for i in range(B0):
        cm = Meta()
        cl = clusters[i].astype(np.int64)
        order = np.argsort(cl, kind="stable")
        sc = cl[order]
        sblk = sc // 128
        idx_g = np.zeros(m.TT, np.int64)           # gather pad -> token 0
        idx_s = np.full(m.TT, NSEQ, np.int64)      # scatter pad -> dump row
        P = np.zeros((m.TT, 128), np.float32)
        for mm in range(NBLK):
            lo = int(np.searchsorted(sblk, mm))
            hi = int(np.searchsorted(sblk, mm + 1))
            cnt = hi - lo
            if cnt == 0:
                continue
            dst = offs[mm] + np.arange(cnt)
            idx_g[dst] = order[lo:hi]
            idx_s[dst] = order[lo:hi]
            P[dst, sc[lo:hi] - 128 * mm] = 1.0
        # P in sbuf layout [128 (token-in-chunk), Tc*128 (chunk, clustercol)]
        cm.P_sb = np.ascontiguousarray(
            P.reshape(m.Tc, 128, 128).transpose(1, 0, 2).reshape(128, m.Tc * 128)
        ).astype(bf16)
        # PT in sbuf layout [128 (clustercol), Tc*128 (chunk, token-in-chunk)]
        cm.PT_sb = np.ascontiguousarray(
            P.reshape(m.Tc, 128, 128).transpose(2, 0, 1).reshape(128, m.Tc * 128)
        ).astype(bf16)
        cm.idx_g = np.ascontiguousarray(np.tile(
            idx_g.reshape(m.TT // 16, 16).T, (8, 1))).astype(np.int16)
        cm.idx_s = np.ascontiguousarray(np.tile(
            idx_s.reshape(m.TT // 16, 16).T, (8, 1))).astype(np.int16)
        cnts = counts[i].astype(np.float64)        # [CPAD]
        w = np.where(cnts > 0, 1.0 / np.maximum(cnts, 1), 0.0)
        lnc = np.where(cnts > 0, np.log(np.maximum(cnts, 1)), LN_NEG)
        cm.wcol = np.ascontiguousarray(
            w.reshape(NBLK, 128).T).astype(np.float32)       # [128, NBLK]
        cm.lncnt = np.ascontiguousarray(
            lnc.reshape(NBLK, 128).T).astype(np.float32)     # [128, NBLK]
        m.cores.append(cm)
    return m


# ------------------------------------------------------------- bass builder

def build_nc(meta: Meta, n_rows: int = ROWS, debug_dump: bool = False,
             stage: int = 99):
    import concourse.bacc as bacc
    import concourse.mybir as mybir
    import concourse.tile as tile
    from concourse import bass
    from concourse.masks import make_identity

    dt = mybir.dt
    Tc = meta.Tc
    TT = meta.TT
    BLK = meta.blk_of_chunk

    nc = bacc.Bacc("TRN2", target_bir_lowering=False, debug=False,
                   num_devices=NCORES)

    q_ext = nc.dram_tensor("q", [n_rows, NSEQ, D], dt.float32, kind="ExternalInput")
    k_ext = nc.dram_tensor("k", [n_rows, NSEQ, D], dt.float32, kind="ExternalInput")
    v_ext = nc.dram_tensor("v", [n_rows, NSEQ, D], dt.float32, kind="ExternalInput")
    P_ext = nc.dram_tensor("P", [128, Tc * 128], dt.bfloat16, kind="ExternalInput")
    PT_ext = nc.dram_tensor("PT", [128, Tc * 128], dt.bfloat16, kind="ExternalInput")
    ig_ext = nc.dram_tensor("idxg", [128, TT // 16], dt.int16, kind="ExternalInput")
    is_ext = nc.dram_tensor("idxs", [128, TT // 16], dt.int16, kind="ExternalInput")
    w_ext = nc.dram_tensor("wcol", [128, NBLK], dt.float32, kind="ExternalInput")
    ln_ext = nc.dram_tensor("lncnt", [128, NBLK], dt.float32, kind="ExternalInput")
    out_ext = nc.dram_tensor("out", [n_rows, NSEQ + 1, D], dt.float32,
                             kind="ExternalOutput")
    a0_ext = nc.dram_tensor("a0", [n_rows, C], dt.float32, kind="ExternalOutput")
    if debug_dump:
        dbg_vaug = nc.dram_tensor("dbg_vaug", [128, NBLK, D + 4], dt.float32,
                                  kind="ExternalOutput")
        dbg_vnrm = nc.dram_tensor("dbg_vnrm", [128, NBLK, D], dt.float32,
                                  kind="ExternalOutput")
        dbg_osort = nc.dram_tensor("dbg_osort", [128, Tc, D], dt.float32,
                                   kind="ExternalOutput")

    Exp = mybir.ActivationFunctionType.Exp

    with tile.TileContext(nc) as tc:
        with (
            tc.tile_pool(name="const", bufs=1) as constp,
            tc.tile_pool(name="gath", bufs=2) as gathp,
            tc.tile_pool(name="gb16", bufs=2) as gb16p,
            tc.tile_pool(name="rowbuf", bufs=2) as rowp,
            tc.tile_pool(name="osort", bufs=2) as osortp,
            tc.tile_pool(name="small", bufs=4) as smallp,
            tc.tile_pool(name="pscd", bufs=3, space="PSUM") as pscdp,
            tc.tile_pool(name="psqk", bufs=2, space="PSUM") as psqkp,
            tc.tile_pool(name="psmall", bufs=3, space="PSUM") as psmallp,
        ):
            # ---- constants
            P_sb = constp.tile([128, Tc * 128], dt.bfloat16)
            PT_sb = constp.tile([128, Tc * 128], dt.bfloat16)
            ig_sb = constp.tile([128, TT // 16], dt.int16)
            is_sb = constp.tile([128, TT // 16], dt.int16)
            w_sb = constp.tile([128, NBLK], dt.float32)
            ln_sb = constp.tile([128, NBLK], dt.float32)
            ones_sb = constp.tile([128, 1], dt.bfloat16)
            ident_sb = constp.tile([128, 128], dt.bfloat16)
            nc.sync.dma_start(P_sb[:, :], P_ext[:, :])
            nc.sync.dma_start(PT_sb[:, :], PT_ext[:, :])
            nc.sync.dma_start(ig_sb[:, :], ig_ext[:, :])
            nc.sync.dma_start(is_sb[:, :], is_ext[:, :])
            nc.sync.dma_start(w_sb[:, :], w_ext[:, :])
            nc.sync.dma_start(ln_sb[:, :], ln_ext[:, :])
            nc.vector.memset(ones_sb[:, :], 1.0)
            make_identity(nc, ident_sb[:, :])

            for r in range(n_rows):
                # ---- gather sorted tokens (f32, 512B rows)
                gt = {}
                for name, ext in (("q", q_ext), ("k", k_ext), ("v", v_ext)):
                    g = gathp.tile([128, Tc, D], dt.float32, tag="gath")
                    nc.gpsimd.dma_gather(
                        out_ap=g[:, :, :],
                        in_ap=ext[r, :, :],
                        idxs_ap=ig_sb[:, :],
                        num_idxs=TT,
                        num_idxs_reg=TT,
                        elem_size=D,
                        single_packet=False,
                    )
                    gt[name] = g

                # ---- cast to bf16
                gb = {}
                for name in ("q", "k", "v"):
                    b = gb16p.tile([128, Tc, D], dt.bfloat16, tag="gb16")
                    nc.vector.tensor_copy(b[:, :, :], gt[name][:, :, :])
                    gb[name] = b

                if stage < 2:
                    nc.sync.dma_start(out_ext[r, 0:128, :],
                                      gt["q"][:, 0, :])
                    continue
                # ---- segment sums (CD layout: [cluster-in-block, D]) + w scale
                qc_cd = rowp.tile([128, NBLK, D], dt.bfloat16, tag="qc")
                kc_cd = rowp.tile([128, NBLK, D], dt.bfloat16, tag="kc")
                v_aug = rowp.tile([128, NBLK, D + 4], dt.bfloat16, tag="vaug")
                for name, dst in (("q", qc_cd), ("k", kc_cd), ("v", v_aug)):
                    for mm in range(NBLK):
                        ps = pscdp.tile([128, D], dt.float32, tag="pscd")
                        for t in range(meta.first_chunk[mm], meta.last_chunk[mm] + 1):
                            nc.tensor.matmul(
                                ps[:, :],
                                lhsT=P_sb[:, bass.ts(t, 128)],
                                rhs=gb[name][:, t, :],
                                start=(t == meta.first_chunk[mm]),
                                stop=(t == meta.last_chunk[mm]),
                            )
                        if name == "v":
                            nc.vector.tensor_scalar_mul(
                                dst[:, mm, 0:D], ps[:, :], w_sb[:, mm:mm + 1])
                        else:
                            nc.vector.tensor_scalar_mul(
                                dst[:, mm, :], ps[:, :], w_sb[:, mm:mm + 1])
                nc.vector.memset(v_aug[:, :, D:D + 1], 1.0)

                if stage < 3:
                    nc.gpsimd.dma_start(out_ext[r, 0:128, :], qc_cd[:, 0, :])
                    continue
                # ---- transpose q_c, k_c to DC layout [D, cluster]
                qdc = rowp.tile([128, NBLK, 128], dt.bfloat16, tag="qdc")
                kdc = rowp.tile([128, NBLK, 128], dt.bfloat16, tag="kdc")
                for src, dst in ((qc_cd, qdc), (kc_cd, kdc)):
                    for mm in range(NBLK):
                        pst = psmallp.tile([128, 128], dt.bfloat16, tag="psmall")
                        nc.tensor.transpose(pst[:, :], src[:, mm, 0:D],
                                            ident_sb[:, :])
                        nc.vector.tensor_copy(dst[:, mm, :], pst[:, :])
                qdc_f = qdc[:, :, :].rearrange("p a b -> p (a b)")

                if stage < 4:
                    nc.gpsimd.dma_start(out_ext[r, 0:128, :], qdc[:, 0, :])
                    continue
                # ---- qkT = k_c q_c^T per e-chunk, then aT = exp(qkT + ln cnt)
                aT = rowp.tile([128, NBLK, 520], dt.bfloat16, tag="aT")
                ps_sA = psmallp.tile([1, 512], dt.float32, tag="psmall")
                ps_sB = psmallp.tile([1, 8], dt.float32, tag="psmall")
                for j in range(NBLK):
                    psA = psqkp.tile([128, 512], dt.float32, tag="psqk")
                    psB = psmallp.tile([128, 8], dt.float32, tag="psmall")
                    nc.tensor.matmul(psA[:, :], lhsT=kdc[:, j, :],
                                     rhs=qdc_f[:, 0:512], start=True, stop=True)
                    nc.tensor.matmul(psB[:, :], lhsT=kdc[:, j, :],
                                     rhs=qdc_f[:, 512:520], start=True, stop=True)
                    nc.scalar.activation(aT[:, j, 0:512], psA[:, :], Exp,
                                         bias=ln_sb[:, j:j + 1])
                    nc.scalar.activation(aT[:, j, 512:520], psB[:, :], Exp,
                                         bias=ln_sb[:, j:j + 1])
                    # rowsum s (free-dim layout, for a0)
                    nc.tensor.matmul(ps_sA[:, :], lhsT=ones_sb[:, :],
                                     rhs=aT[:, j, 0:512],
                                     start=(j == 0), stop=(j == NBLK - 1))
                    nc.tensor.matmul(ps_sB[:, :], lhsT=ones_sb[:, :],
                                     rhs=aT[:, j, 512:520],
                                     start=(j == 0), stop=(j == NBLK - 1))

                if stage < 5:
                    nc.gpsimd.dma_start(out_ext[r, 0:128, :], aT[:, 0, 0:D])
                    continue
                # ---- a0 = aT[0, :] / s
                rA = smallp.tile([1, 512], dt.float32, tag="ra")
                rB = smallp.tile([1, 8], dt.float32, tag="rb")
                a0_sb = smallp.tile([1, 520], dt.float32, tag="a0")
                nc.vector.reciprocal(rA[:, :], ps_sA[:, :])
                nc.vector.reciprocal(rB[:, :], ps_sB[:, :])
                nc.vector.tensor_mul(a0_sb[:, 0:512], aT[0:1, 0, 0:512], rA[:, :])
                nc.vector.tensor_mul(a0_sb[:, 512:520], aT[0:1, 0, 512:520], rB[:, :])
                nc.sync.dma_start(a0_ext[r:r + 1, :], a0_sb[0:1, 0:C])

                if stage < 6:
                    continue
                # ---- v_out = aT^T @ [v_c | 1]; normalize by rowsum col
                v_nrm = rowp.tile([128, NBLK, D], dt.bfloat16, tag="vnrm")
                for i in range(NBLK):
                    M = 128 if i < NBLK - 1 else (C - 128 * (NBLK - 1))
                    psv = psmallp.tile([M, D + 4], dt.float32, tag="psmall")
                    for j in range(NBLK):
                        nc.tensor.matmul(
                            psv[:, 0:D + 1],
                            lhsT=aT[:, j, 128 * i:128 * i + M],
                            rhs=v_aug[:, j, 0:D + 1],
                            start=(j == 0), stop=(j == NBLK - 1),
                        )
                    rv = smallp.tile([M, 1], dt.float32, tag="rv")
                    nc.vector.reciprocal(rv[:, :], psv[:, D:D + 1])
                    if M < 128:
                        nc.vector.memset(v_nrm[:, i, :], 0.0)
                    nc.vector.tensor_scalar_mul(v_nrm[0:M, i, :], psv[:, 0:D], rv[:, :])

                if stage < 7:
                    nc.gpsimd.dma_start(out_ext[r, 0:128, :], v_nrm[:, 0, :])
                    continue
                # ---- broadcast back to (sorted) tokens, scatter to out
                osort = osortp.tile([128, Tc, D], dt.float32, tag="osort")
                for t in range(Tc):
                    pso = psmallp.tile([128, D], dt.float32, tag="psmall")
                    nc.tensor.matmul(pso[:, :], lhsT=PT_sb[:, bass.ts(t, 128)],
                                     rhs=v_nrm[:, BLK[t], :], start=True, stop=True)
                    nc.scalar.copy(osort[:, t, :], pso[:, :])
                nc.gpsimd.dma_scatter_add(
                    out_ext[r, :, :],
                    osort[:, :, :],
                    is_sb[:, :],
                    TT,
                    TT,
                    D,
                    single_packet=False,
                )
                if debug_dump and r == 0:
                    dbg_vaug_sb = rowp.tile([128, NBLK, D + 4], dt.float32,
                                            tag="dbgva")
                    nc.vector.tensor_copy(dbg_vaug_sb[:, :, 0:D + 1], v_aug[:, :, 0:D + 1])
                    nc.vector.memset(dbg_vaug_sb[:, :, D + 1:D + 4], 0.0)
                    nc.sync.dma_start(dbg_vaug[:, :, :], dbg_vaug_sb[:, :, :])
                    dbg_vnrm_sb = rowp.tile([128, NBLK, D], dt.float32,
                                            tag="dbgvn")
                    nc.vector.tensor_copy(dbg_vnrm_sb[:, :, :], v_nrm[:, :, :])
                    nc.sync.dma_start(dbg_vnrm[:, :, :], dbg_vnrm_sb[:, :, :])
                    nc.sync.dma_start(dbg_osort[:, :, :], osort[:, :, :])

    return nc


# ------------------------------------------------------------------- runner

_CACHE = {}


def _ensure_ntff_hook():
    """Register the NTFF profiling hook (absent antenv.axon_hooks stub)."""
    import types
    if "antenv.axon_hooks" in sys.modules:
        return
    m = types.ModuleType("antenv.axon_hooks")
    m._hook = None
    m.set_axon_ntff_profile_hook = lambda h: setattr(m, "_hook", h)
    m.get_axon_ntff_profile_hook = lambda: m._hook
    sys.modules["antenv.axon_hooks"] = m
    try:
        import antenv
        antenv.axon_hooks = m
    except Exception:
        pass
    try:
        from trn_agent_boot.trn_boot import _ntff_profile_via_ctypes
        hook = _ntff_profile_via_ctypes("/opt/axon/libaxon_pjrt.so")
        if hook is not None:
            m._hook = hook
    except Exception:
        pass


def make_in_maps(queries, keys, values, meta, n_rows=ROWS):
    in_maps = []
    for i in range(NCORES):
        cm = meta.cores[i]
        in_maps.append({
            "q": np.ascontiguousarray(queries[i::NCORES][:n_rows], np.float32),
            "k": np.ascontiguousarray(keys[i::NCORES][:n_rows], np.float32),
            "v": np.ascontiguousarray(values[i::NCORES][:n_rows], np.float32),
            "P": cm.P_sb,
            "PT": cm.PT_sb,
            "idxg": cm.idx_g,
            "idxs": cm.idx_s,
            "wcol": cm.wcol,
            "lncnt": cm.lncnt,
        })
    return in_maps


def run_cores(queries, keys, values, clusters, trace=False, n_rows=ROWS):
    _ensure_ntff_hook()
    from concourse.bass_utils import run_bass_kernel_spmd

    meta = build_meta(np.asarray(clusters))
    key = (meta.T_m, n_rows)
    if key not in _CACHE:
        nc = build_nc(meta, n_rows)
        nc.finalize()
        _CACHE[key] = nc
    nc = _CACHE[key]
    in_maps = make_in_maps(np.asarray(queries), np.asarray(keys),
                           np.asarray(values), meta, n_rows)
    res = run_bass_kernel_spmd(nc, in_maps, core_ids=list(range(NCORES)),
                               trace=trace)
    return res, meta


def kernel(**inputs):
    queries = np.asarray(inputs["queries"], np.float32)
    keys = np.asarray(inputs["keys"], np.float32)
    values = np.asarray(inputs["values"], np.float32)
    clusters = np.asarray(inputs["clusters"], np.int32)

    res, _ = run_cores(queries, keys, values, clusters, trace=False)

    out = np.empty((B, NSEQ, D), np.float32)
    a0 = np.empty((B, C), np.float32)
    for i in range(NCORES):
        r = res.results[i]
        out[i::NCORES] = r["out"][:, :NSEQ, :]
        a0[i::NCORES] = r["a0"]
    return out, a0


# revision 17
# speedup vs baseline: 3.2318x; 3.2318x over previous
"""AdaClusteringAttention Trainium2 kernel (8 NeuronCores, batch/head parallel).

Reference semantics (per batch*head row b, cluster row = clusters[b % 8]):
  q_c/k_c/v_c = per-cluster means (segment-sum * 1/count)      [C=513, D=128]
  qk = q_c @ k_c^T ; a = softmax(qk) * counts ; a /= rowsum    [C, C]
  v  = a @ v_c ; out[n] = v[cluster[n]] ; a0 = a[:, 0]

Device strategy per core (8 rows each, all sharing ONE cluster row):
  - host: stable-sort tokens by cluster; pad each 128-cluster block's token
    list to a multiple of 128 (padding uniform across cores => one SPMD graph)
  - host packs q/k/v for the core's 8 rows token-major into one bf16 tensor
    qkv8[n, (tensor,row,d)] so a single dma_gather descriptor moves 6KB per
    token (Q7 descriptor generation is the scarce resource)
  - segment sums = per-128-token-chunk matmuls against one-hot blocks
    (a chunk's tokens all fall inside one 128-cluster block); row pairs are
    adjacent in the gathered layout so matmuls run at N=256
  - cluster attention: qkT = k_c q_c^T, aT = exp(qkT + ln(count[e]))
    (count-weighted softmax; max-subtraction skipped, scale cancels)
  - v_out = aT^T @ [v_c | 1] gives numerator and rowsum together
  - out tokens via one-hot-transpose matmuls in sorted order into a
    row-grouped buffer, then dma_scatter_add (4KB/token) back to token
    order (outputs are zero-initialized; pads go to dump row NSEQ)
"""

import sys

import numpy as np

B0, H, NSEQ, D = 8, 8, 4096, 128
B = B0 * H
C = 513
NBLK = 5            # ceil(C/128) cluster blocks
CPAD = NBLK * 128   # 640
NCORES = 8
ROWS = B // NCORES  # 8 rows per core
LN_NEG = -88.0      # exp(-88) == 0 in f32/bf16
OPTOK = 512         # tokens per gather/scatter op (SBUF footprint knob)


def _bf16():
    import ml_dtypes
    return ml_dtypes.bfloat16


# ----------------------------------------------------------------- host meta

class Meta:
    pass


def build_meta(clusters: np.ndarray) -> Meta:
    m = Meta()
    assert clusters.shape == (B0, NSEQ)
    counts = np.zeros((B0, CPAD), np.int64)
    for i in range(B0):
        counts[i, :C] = np.bincount(clusters[i], minlength=C)
    blk_tok = counts.reshape(B0, NBLK, 128).sum(-1)
    T_m = np.maximum(128, (np.ceil(blk_tok.max(0) / 128) * 128).astype(np.int64))
    m.T_m = tuple(int(x) for x in T_m)
    m.TT = int(T_m.sum())
    m.Tc = m.TT // 128
    offs = np.concatenate([[0], np.cumsum(T_m)]).astype(np.int64)
    m.offs = offs
    blk_of_chunk = []
    for mm in range(NBLK):
        blk_of_chunk += [mm] * (m.T_m[mm] // 128)
    m.blk_of_chunk = tuple(blk_of_chunk)
    first, last = {}, {}
    for t, mm in enumerate(m.blk_of_chunk):
        first.setdefault(mm, t)
        last[mm] = t
    m.first_chunk = first
    m.last_chunk = last

    bf16 = _bf16()
    m.cores = []
    for i in range(B0):
        cm = Meta()
        cl = clusters[i].astype(np.int64)
        order = np.argsort(cl, kind="stable")
        sc = cl[order]
        sblk = sc // 128
        idx_g = np.zeros(m.TT, np.int64)           # gather pad -> token 0
        idx_s = np.full(m.TT, NSEQ, np.int64)      # scatter pad -> dump row
        P = np.zeros((m.TT, 128), np.float32)
        for mm in range(NBLK):
            lo = int(np.searchsorted(sblk, mm))
            hi = int(np.searchsorted(sblk, mm + 1))
            if hi == lo:
                continue
            dst = offs[mm] + np.arange(hi - lo)
            idx_g[dst] = order[lo:hi]
            idx_s[dst] = order[lo:hi]
            P[dst, sc[lo:hi] - 128 * mm] = 1.0
        cm.P_sb = np.ascontiguousarray(
            P.reshape(m.Tc, 128, 128).transpose(1, 0, 2).reshape(128, m.Tc * 128)
        ).astype(bf16)
        cm.PT_sb = np.ascontiguousarray(
            P.reshape(m.Tc, 128, 128).transpose(2, 0, 1).reshape(128, m.Tc * 128)
        ).astype(bf16)
        cm.idx_g = np.ascontiguousarray(np.tile(
            idx_g.reshape(m.TT // 16, 16).T, (8, 1))).astype(np.int16)
        cm.idx_s = np.ascontiguousarray(np.tile(
            idx_s.reshape(m.TT // 16, 16).T, (8, 1))).astype(np.int16)
        cnts = counts[i].astype(np.float64)
        w = np.where(cnts > 0, 1.0 / np.maximum(cnts, 1), 0.0)
        lnc = np.where(cnts > 0, np.log(np.maximum(cnts, 1)), LN_NEG)
        cm.wcol = np.ascontiguousarray(
            w.reshape(NBLK, 128).T).astype(np.float32)
        cm.lncnt = np.ascontiguousarray(
            lnc.reshape(NBLK, 128).T).astype(np.float32)
        m.cores.append(cm)
    return m


# ------------------------------------------------------------- bass builder

def build_nc(meta: Meta, n_rows: int = ROWS):
    import concourse.bacc as bacc
    import concourse.mybir as mybir
    import concourse.tile as tile
    from concourse import bass
    from concourse.masks import make_identity

    dt = mybir.dt
    Tc, TT, BLK = meta.Tc, meta.TT, meta.blk_of_chunk
    EW = 3 * n_rows * D           # gathered row width (elems, bf16)
    OW = n_rows * D               # out8 row width (elems, f32)
    n_ops = (TT + OPTOK - 1) // OPTOK
    op_tok = [min(OPTOK, TT - c * OPTOK) for c in range(n_ops)]

    nc = bacc.Bacc("TRN2", target_bir_lowering=False, debug=False,
                   num_devices=NCORES)

    qkv_ext = nc.dram_tensor("qkv", [NSEQ, EW], dt.bfloat16, kind="ExternalInput")
    P_ext = nc.dram_tensor("P", [128, Tc * 128], dt.bfloat16, kind="ExternalInput")
    PT_ext = nc.dram_tensor("PT", [128, Tc * 128], dt.bfloat16, kind="ExternalInput")
    ig_ext = nc.dram_tensor("idxg", [128, TT // 16], dt.int16, kind="ExternalInput")
    is_ext = nc.dram_tensor("idxs", [128, TT // 16], dt.int16, kind="ExternalInput")
    w_ext = nc.dram_tensor("wcol", [128, NBLK], dt.float32, kind="ExternalInput")
    ln_ext = nc.dram_tensor("lncnt", [128, NBLK], dt.float32, kind="ExternalInput")
    out_ext = nc.dram_tensor("out8", [NSEQ + 1, OW], dt.float32,
                             kind="ExternalOutput")
    a0_ext = nc.dram_tensor("a0", [n_rows, C], dt.float32, kind="ExternalOutput")

    Exp = mybir.ActivationFunctionType.Exp
    QW = min(4, n_rows)           # rows per segsum matmul (one PSUM bank)
    nquad = n_rows // QW

    with tile.TileContext(nc) as tc:
        with (
            tc.tile_pool(name="const", bufs=1) as constp,
            tc.tile_pool(name="gath", bufs=2) as gathp,
            tc.tile_pool(name="rowbuf", bufs=2) as rowp,
            tc.tile_pool(name="persist", bufs=1) as perp,
            tc.tile_pool(name="osort", bufs=2) as osortp,
            tc.tile_pool(name="small", bufs=4) as smallp,
        ):
            # ---- constants
            P_sb = constp.tile([128, Tc * 128], dt.bfloat16)
            PT_sb = constp.tile([128, Tc * 128], dt.bfloat16)
            ig_sb = constp.tile([128, TT // 16], dt.int16)
            is_sb = constp.tile([128, TT // 16], dt.int16)
            w_sb = constp.tile([128, NBLK], dt.float32)
            ln_sb = constp.tile([128, NBLK], dt.float32)
            ones_sb = constp.tile([128, 1], dt.bfloat16)
            ident_sb = constp.tile([128, 128], dt.bfloat16)
            nc.sync.dma_start(P_sb[:, :], P_ext[:, :])
            nc.sync.dma_start(PT_sb[:, :], PT_ext[:, :])
            nc.sync.dma_start(ig_sb[:, :], ig_ext[:, :])
            nc.sync.dma_start(is_sb[:, :], is_ext[:, :])
            nc.sync.dma_start(w_sb[:, :], w_ext[:, :])
            nc.sync.dma_start(ln_sb[:, :], ln_ext[:, :])
            nc.vector.memset(ones_sb[:, :], 1.0)
            make_identity(nc, ident_sb[:, :])

            # ---- persistent per-row results
            qc_cd = perp.tile([128, n_rows, NBLK, D], dt.bfloat16)
            kc_cd = perp.tile([128, n_rows, NBLK, D], dt.bfloat16)
            v_aug = perp.tile([128, n_rows, NBLK, D + 4], dt.bfloat16)
            v_nrm = perp.tile([128, n_rows, NBLK, D], dt.bfloat16)
            nc.vector.memset(v_aug[:, :, :, D:D + 1], 1.0)

            # ---- phase 1: gather + segment sums for all rows at once
            ph1 = tc.tile_pool(name="pscd", bufs=1, space="PSUM")
            pscdp = ph1.__enter__()
            ps_cd = {}
            for x in range(3):
                ps_cd[x] = pscdp.tile([128, n_rows * D], dt.float32,
                                      tag=f"cd{x}", name=f"ps_cd{x}")
            gtiles = []
            for c in range(n_ops):
                g = gathp.tile([128, OPTOK // 128, EW], dt.bfloat16, tag="gath")
                nc.gpsimd.dma_gather(
                    out_ap=g[:, 0:op_tok[c] // 128, :],
                    in_ap=qkv_ext[:, :],
                    idxs_ap=ig_sb[:, c * (OPTOK // 16):
                                  c * (OPTOK // 16) + op_tok[c] // 16],
                    num_idxs=op_tok[c],
                    num_idxs_reg=op_tok[c],
                    elem_size=EW,
                    single_packet=False,
                )
                gtiles.append(g)

            for t in range(Tc):
                mm = BLK[t]
                c, t_loc = t // (OPTOK // 128), t % (OPTOK // 128)
                g = gtiles[c]
                for x in range(3):
                    for q in range(nquad):
                        nc.tensor.matmul(
                            ps_cd[x][:, bass.ts(q, QW * D)],
                            lhsT=P_sb[:, bass.ts(t, 128)],
                            rhs=g[:, t_loc,
                                  (x * n_rows + QW * q) * D:
                                  (x * n_rows + QW * (q + 1)) * D],
                            start=(t == meta.first_chunk[mm]),
                            stop=(t == meta.last_chunk[mm]),
                        )
                if t == meta.last_chunk[mm]:
                    for x, dst in ((0, qc_cd), (1, kc_cd), (2, v_aug)):
                        nc.vector.tensor_scalar_mul(
                            dst[:, :, mm, 0:D], ps_cd[x][:, :],
                            w_sb[:, mm:mm + 1])

            ph1.__exit__(None, None, None)

            # ---- phase 2/3 PSUM pools
            ph2a = tc.tile_pool(name="psqk", bufs=2, space="PSUM")
            psqkp = ph2a.__enter__()
            ph2b = tc.tile_pool(name="psmall", bufs=4, space="PSUM")
            psmallp = ph2b.__enter__()

            # ---- phase 2: per-row cluster attention
            for r in range(n_rows):
                qdc = rowp.tile([128, NBLK, 128], dt.bfloat16, tag="qdc")
                kdc = rowp.tile([128, NBLK, 128], dt.bfloat16, tag="kdc")
                for src, dst in ((qc_cd, qdc), (kc_cd, kdc)):
                    for mm in range(NBLK):
                        pst = psmallp.tile([128, 128], dt.bfloat16, tag="psmall")
                        nc.tensor.transpose(pst[:, :], src[:, r, mm, 0:D],
                                            ident_sb[:, :])
                        nc.vector.tensor_copy(dst[:, mm, :], pst[:, :])
                qdc_f = qdc[:, :, :].rearrange("p a b -> p (a b)")

                aT = rowp.tile([128, NBLK, 520], dt.bfloat16, tag="aT")
                ps_sA = psmallp.tile([1, 512], dt.float32, tag="psmall")
                ps_sB = psmallp.tile([1, 8], dt.float32, tag="psmall")
                for j in range(NBLK):
                    psA = psqkp.tile([128, 512], dt.float32, tag="psqk")
                    psB = psmallp.tile([128, 8], dt.float32, tag="psmall")
                    nc.tensor.matmul(psA[:, :], lhsT=kdc[:, j, :],
                                     rhs=qdc_f[:, 0:512], start=True, stop=True)
                    nc.tensor.matmul(psB[:, :], lhsT=kdc[:, j, :],
                                     rhs=qdc_f[:, 512:520], start=True, stop=True)
                    nc.scalar.activation(aT[:, j, 0:512], psA[:, :], Exp,
                                         bias=ln_sb[:, j:j + 1])
                    nc.scalar.activation(aT[:, j, 512:520], psB[:, :], Exp,
                                         bias=ln_sb[:, j:j + 1])
                    nc.tensor.matmul(ps_sA[:, :], lhsT=ones_sb[:, :],
                                     rhs=aT[:, j, 0:512],
                                     start=(j == 0), stop=(j == NBLK - 1))
                    nc.tensor.matmul(ps_sB[:, :], lhsT=ones_sb[:, :],
                                     rhs=aT[:, j, 512:520],
                                     start=(j == 0), stop=(j == NBLK - 1))

                rA = smallp.tile([1, 512], dt.float32, tag="ra")
                rB = smallp.tile([1, 8], dt.float32, tag="rb")
                a0_sb = smallp.tile([1, 520], dt.float32, tag="a0")
                nc.vector.reciprocal(rA[:, :], ps_sA[:, :])
                nc.vector.reciprocal(rB[:, :], ps_sB[:, :])
                nc.vector.tensor_mul(a0_sb[:, 0:512], aT[0:1, 0, 0:512], rA[:, :])
                nc.vector.tensor_mul(a0_sb[:, 512:520], aT[0:1, 0, 512:520],
                                     rB[:, :])
                nc.sync.dma_start(a0_ext[r:r + 1, :], a0_sb[0:1, 0:C])

                for i in range(NBLK):
                    M = 128 if i < NBLK - 1 else (C - 128 * (NBLK - 1))
                    psv = psmallp.tile([M, D + 4], dt.float32, tag="psmall")
                    for j in range(NBLK):
                        nc.tensor.matmul(
                            psv[:, 0:D + 1],
                            lhsT=aT[:, j, 128 * i:128 * i + M],
                            rhs=v_aug[:, r, j, 0:D + 1],
                            start=(j == 0), stop=(j == NBLK - 1),
                        )
                    rv = smallp.tile([M, 1], dt.float32, tag="rv")
                    nc.vector.reciprocal(rv[:, :], psv[:, D:D + 1])
                    if M < 128:
                        nc.vector.memset(v_nrm[:, r, i, :], 0.0)
                    nc.vector.tensor_scalar_mul(v_nrm[0:M, r, i, :],
                                                psv[:, 0:D], rv[:, :])

            # ---- phase 3: broadcast to sorted tokens + scatter (row-grouped)
            for c in range(n_ops):
                ntok = op_tok[c]
                osort = osortp.tile([128, OPTOK // 128, OW], dt.float32,
                                    tag="osort")
                for t_loc in range(ntok // 128):
                    t = c * (OPTOK // 128) + t_loc
                    for r in range(n_rows):
                        pso = psmallp.tile([128, D], dt.float32, tag="psmall")
                        nc.tensor.matmul(pso[:, :],
                                         lhsT=PT_sb[:, bass.ts(t, 128)],
                                         rhs=v_nrm[:, r, BLK[t], :],
                                         start=True, stop=True)
                        if (t_loc + r) % 2:
                            nc.scalar.copy(osort[:, t_loc, r * D:(r + 1) * D],
                                           pso[:, :])
                        else:
                            nc.vector.tensor_copy(
                                osort[:, t_loc, r * D:(r + 1) * D], pso[:, :])
                nc.gpsimd.dma_scatter_add(
                    out_ext[:, :],
                    osort[:, 0:ntok // 128, :],
                    is_sb[:, c * (OPTOK // 16):
                          c * (OPTOK // 16) + ntok // 16],
                    ntok,
                    ntok,
                    OW,
                    single_packet=False,
                )
            ph2b.__exit__(None, None, None)
            ph2a.__exit__(None, None, None)

    return nc


# ------------------------------------------------------------------- runner

_CACHE = {}


def _ensure_ntff_hook():
    import types
    if "antenv.axon_hooks" in sys.modules:
        return
    m = types.ModuleType("antenv.axon_hooks")
    m._hook = None
    m.set_axon_ntff_profile_hook = lambda h: setattr(m, "_hook", h)
    m.get_axon_ntff_profile_hook = lambda: m._hook
    sys.modules["antenv.axon_hooks"] = m
    try:
        import antenv
        antenv.axon_hooks = m
    except Exception:
        pass
    try:
        from trn_agent_boot.trn_boot import _ntff_profile_via_ctypes
        hook = _ntff_profile_via_ctypes("/opt/axon/libaxon_pjrt.so")
        if hook is not None:
            m._hook = hook
    except Exception:
        pass


def make_in_maps(queries, keys, values, meta, n_rows=ROWS):
    bf16 = _bf16()
    in_maps = []
    for i in range(NCORES):
        cm = meta.cores[i]
        # token-major pack: qkv8[n, x*n_rows + r, :] = X_x[i + 8r, n, :]
        qkv = np.empty((NSEQ, 3, n_rows, D), dtype=bf16)
        for x, src in enumerate((queries, keys, values)):
            rows = src[i::NCORES][:n_rows]          # [n_rows, NSEQ, D]
            qkv[:, x, :, :] = rows.transpose(1, 0, 2).astype(bf16)
        in_maps.append({
            "qkv": qkv.reshape(NSEQ, 3 * n_rows * D),
            "P": cm.P_sb,
            "PT": cm.PT_sb,
            "idxg": cm.idx_g,
            "idxs": cm.idx_s,
            "wcol": cm.wcol,
            "lncnt": cm.lncnt,
        })
    return in_maps


def run_cores(queries, keys, values, clusters, trace=False, n_rows=ROWS):
    _ensure_ntff_hook()
    from concourse.bass_utils import run_bass_kernel_spmd

    meta = build_meta(np.asarray(clusters))
    key = (meta.T_m, n_rows)
    if key not in _CACHE:
        nc = build_nc(meta, n_rows)
        nc.finalize()
        _CACHE[key] = nc
    nc = _CACHE[key]
    in_maps = make_in_maps(np.asarray(queries), np.asarray(keys),
                           np.asarray(values), meta, n_rows)
    res = run_bass_kernel_spmd(nc, in_maps, core_ids=list(range(NCORES)),
                               trace=trace)
    return res, meta


def unshard(res, n_rows=ROWS):
    out = np.empty((B, NSEQ, D), np.float32)
    a0 = np.empty((B, C), np.float32)
    for i in range(NCORES):
        r = res.results[i]
        o8 = r["out8"][:NSEQ].reshape(NSEQ, n_rows, D)
        for rr in range(n_rows):
            out[i + NCORES * rr] = o8[:, rr, :]
        a0[i::NCORES] = r["a0"][:n_rows]
    return out, a0


def kernel(**inputs):
    queries = np.asarray(inputs["queries"], np.float32)
    keys = np.asarray(inputs["keys"], np.float32)
    values = np.asarray(inputs["values"], np.float32)
    clusters = np.asarray(inputs["clusters"], np.int32)
    res, _ = run_cores(queries, keys, values, clusters, trace=False)
    return unshard(res)


# revision 19
# speedup vs baseline: 3.7109x; 1.1483x over previous
"""AdaClusteringAttention Trainium2 kernel (8 NeuronCores, batch/head parallel).

Reference semantics (per batch*head row b, cluster row = clusters[b % 8]):
  q_c/k_c/v_c = per-cluster means (segment-sum * 1/count)      [C=513, D=128]
  qk = q_c @ k_c^T ; a = softmax(qk) * counts ; a /= rowsum    [C, C]
  v  = a @ v_c ; out[n] = v[cluster[n]] ; a0 = a[:, 0]

Device strategy per core (8 rows each, all sharing ONE cluster row):
  - host: stable-sort tokens by cluster; pad each 128-cluster block's token
    list to a multiple of 128 (padding uniform across cores => one SPMD graph)
  - host packs q/k/v for the core's 8 rows token-major into one bf16 tensor
    qkv8[n, (tensor,row,d)] so a single dma_gather descriptor moves 6KB per
    token (Q7 descriptor generation is the scarce resource)
  - segment sums = per-128-token-chunk matmuls against one-hot blocks
    (a chunk's tokens all fall inside one 128-cluster block); row pairs are
    adjacent in the gathered layout so matmuls run at N=256
  - cluster attention: qkT = k_c q_c^T, aT = exp(qkT + ln(count[e]))
    (count-weighted softmax; max-subtraction skipped, scale cancels)
  - v_out = aT^T @ [v_c | 1] gives numerator and rowsum together
  - out tokens via one-hot-transpose matmuls in sorted order into a
    row-grouped buffer, then dma_scatter_add (4KB/token) back to token
    order (outputs are zero-initialized; pads go to dump row NSEQ)
"""

import sys

import numpy as np

B0, H, NSEQ, D = 8, 8, 4096, 128
B = B0 * H
C = 513
NBLK = 5            # ceil(C/128) cluster blocks
CPAD = NBLK * 128   # 640
NCORES = 8
ROWS = B // NCORES  # 8 rows per core
LN_NEG = -88.0      # exp(-88) == 0 in f32/bf16
OPTOK = 512         # tokens per gather/scatter op (SBUF footprint knob)


def _bf16():
    import ml_dtypes
    return ml_dtypes.bfloat16


# ----------------------------------------------------------------- host meta

class Meta:
    pass


def build_meta(clusters: np.ndarray) -> Meta:
    m = Meta()
    assert clusters.shape == (B0, NSEQ)
    counts = np.zeros((B0, CPAD), np.int64)
    for i in range(B0):
        counts[i, :C] = np.bincount(clusters[i], minlength=C)
    blk_tok = counts.reshape(B0, NBLK, 128).sum(-1)
    T_m = np.maximum(128, (np.ceil(blk_tok.max(0) / 128) * 128).astype(np.int64))
    m.T_m = tuple(int(x) for x in T_m)
    m.TT = int(T_m.sum())
    m.Tc = m.TT // 128
    offs = np.concatenate([[0], np.cumsum(T_m)]).astype(np.int64)
    m.offs = offs
    blk_of_chunk = []
    for mm in range(NBLK):
        blk_of_chunk += [mm] * (m.T_m[mm] // 128)
    m.blk_of_chunk = tuple(blk_of_chunk)
    first, last = {}, {}
    for t, mm in enumerate(m.blk_of_chunk):
        first.setdefault(mm, t)
        last[mm] = t
    m.first_chunk = first
    m.last_chunk = last

    bf16 = _bf16()
    m.cores = []
    for i in range(B0):
        cm = Meta()
        cl = clusters[i].astype(np.int64)
        order = np.argsort(cl, kind="stable")
        sc = cl[order]
        sblk = sc // 128
        idx_g = np.zeros(m.TT, np.int64)           # gather pad -> token 0
        idx_s = np.full(m.TT, NSEQ, np.int64)      # scatter pad -> dump row
        P = np.zeros((m.TT, 128), np.float32)
        for mm in range(NBLK):
            lo = int(np.searchsorted(sblk, mm))
            hi = int(np.searchsorted(sblk, mm + 1))
            if hi == lo:
                continue
            dst = offs[mm] + np.arange(hi - lo)
            idx_g[dst] = order[lo:hi]
            idx_s[dst] = order[lo:hi]
            P[dst, sc[lo:hi] - 128 * mm] = 1.0
        cm.P_sb = np.ascontiguousarray(
            P.reshape(m.Tc, 128, 128).transpose(1, 0, 2).reshape(128, m.Tc * 128)
        ).astype(bf16)
        cm.PT_sb = np.ascontiguousarray(
            P.reshape(m.Tc, 128, 128).transpose(2, 0, 1).reshape(128, m.Tc * 128)
        ).astype(bf16)
        cm.idx_g = np.ascontiguousarray(np.tile(
            idx_g.reshape(m.TT // 16, 16).T, (8, 1))).astype(np.int16)
        cm.idx_s = np.ascontiguousarray(np.tile(
            idx_s.reshape(m.TT // 16, 16).T, (8, 1))).astype(np.int16)
        cnts = counts[i].astype(np.float64)
        w = np.where(cnts > 0, 1.0 / np.maximum(cnts, 1), 0.0)
        lnc = np.where(cnts > 0, np.log(np.maximum(cnts, 1)), LN_NEG)
        cm.wcol = np.ascontiguousarray(
            w.reshape(NBLK, 128).T).astype(np.float32)
        cm.lncnt = np.ascontiguousarray(
            lnc.reshape(NBLK, 128).T).astype(np.float32)
        m.cores.append(cm)
    return m


# ------------------------------------------------------------- bass builder

def build_nc(meta: Meta, n_rows: int = ROWS):
    import concourse.bacc as bacc
    import concourse.mybir as mybir
    import concourse.tile as tile
    from concourse import bass
    from concourse.masks import make_identity

    dt = mybir.dt
    Tc, TT, BLK = meta.Tc, meta.TT, meta.blk_of_chunk
    EW = 3 * n_rows * D           # gathered row width (elems, bf16)
    OW = n_rows * D               # out8 row width (elems, f32)
    n_ops = (TT + OPTOK - 1) // OPTOK
    op_tok = [min(OPTOK, TT - c * OPTOK) for c in range(n_ops)]

    nc = bacc.Bacc("TRN2", target_bir_lowering=False, debug=False,
                   num_devices=NCORES)

    qkv_ext = nc.dram_tensor("qkv", [NSEQ, EW], dt.bfloat16, kind="ExternalInput")
    P_ext = nc.dram_tensor("P", [128, Tc * 128], dt.bfloat16, kind="ExternalInput")
    PT_ext = nc.dram_tensor("PT", [128, Tc * 128], dt.bfloat16, kind="ExternalInput")
    ig_ext = nc.dram_tensor("idxg", [128, TT // 16], dt.int16, kind="ExternalInput")
    is_ext = nc.dram_tensor("idxs", [128, TT // 16], dt.int16, kind="ExternalInput")
    w_ext = nc.dram_tensor("wcol", [128, NBLK], dt.float32, kind="ExternalInput")
    ln_ext = nc.dram_tensor("lncnt", [128, NBLK], dt.float32, kind="ExternalInput")
    HR = max(n_rows // 2, 1)      # rows per output half
    outA_ext = nc.dram_tensor("outA", [NSEQ + 1, HR * D], dt.float32,
                              kind="ExternalOutput")
    outB_ext = nc.dram_tensor("outB", [NSEQ + 1, (n_rows - HR) * D or D],
                              dt.float32, kind="ExternalOutput")
    a0_ext = nc.dram_tensor("a0T", [128, NBLK * n_rows], dt.float32,
                            kind="ExternalOutput")

    Exp = mybir.ActivationFunctionType.Exp
    QW = min(4, n_rows)           # rows per segsum matmul (one PSUM bank)
    nquad = n_rows // QW

    with tile.TileContext(nc) as tc:
        with (
            tc.tile_pool(name="const", bufs=1) as constp,
            tc.tile_pool(name="gath", bufs=2) as gathp,
            tc.tile_pool(name="rowbuf", bufs=2) as rowp,
            tc.tile_pool(name="persist", bufs=1) as perp,
            tc.tile_pool(name="osort", bufs=2) as osortp,
            tc.tile_pool(name="small", bufs=4) as smallp,
        ):
            # ---- constants
            P_sb = constp.tile([128, Tc * 128], dt.bfloat16)
            PT_sb = constp.tile([128, Tc * 128], dt.bfloat16)
            ig_sb = constp.tile([128, TT // 16], dt.int16)
            is_sb = constp.tile([128, TT // 16], dt.int16)
            w_sb = constp.tile([128, NBLK], dt.float32)
            ln_sb = constp.tile([128, NBLK], dt.float32)
            ident_sb = constp.tile([128, 128], dt.bfloat16)
            nc.sync.dma_start(P_sb[:, :], P_ext[:, :])
            nc.sync.dma_start(PT_sb[:, :], PT_ext[:, :])
            nc.sync.dma_start(ig_sb[:, :], ig_ext[:, :])
            nc.sync.dma_start(is_sb[:, :], is_ext[:, :])
            nc.sync.dma_start(w_sb[:, :], w_ext[:, :])
            nc.sync.dma_start(ln_sb[:, :], ln_ext[:, :])
            make_identity(nc, ident_sb[:, :])

            # ---- persistent per-row results
            qc_cd = perp.tile([128, n_rows, NBLK, D], dt.bfloat16)
            kc_cd = perp.tile([128, n_rows, NBLK, D], dt.bfloat16)
            v_aug = perp.tile([128, n_rows, NBLK, D + 4], dt.bfloat16)
            v_nrm = perp.tile([128, n_rows, NBLK, D], dt.bfloat16)
            a0_all = perp.tile([128, NBLK, n_rows], dt.float32)
            nc.vector.memset(a0_all[:, :, :], 0.0)
            nc.vector.memset(v_aug[:, :, :, D:D + 1], 1.0)
            # e0 column: picks out aT[0, :] inside the av matmul (e-chunk 0)
            nc.vector.memset(v_aug[:, :, :, D + 1:D + 2], 0.0)
            nc.vector.memset(v_aug[0:1, :, 0:1, D + 1:D + 2], 1.0)

            # ---- phase 1: gather + segment sums for all rows at once
            ph1 = tc.tile_pool(name="pscd", bufs=1, space="PSUM")
            pscdp = ph1.__enter__()
            ps_cd = {}
            for x in range(3):
                ps_cd[x] = pscdp.tile([128, n_rows * D], dt.float32,
                                      tag=f"cd{x}", name=f"ps_cd{x}")
            gtiles = []
            for c in range(n_ops):
                g = gathp.tile([128, OPTOK // 128, EW], dt.bfloat16, tag="gath")
                nc.gpsimd.dma_gather(
                    out_ap=g[:, 0:op_tok[c] // 128, :],
                    in_ap=qkv_ext[:, :],
                    idxs_ap=ig_sb[:, c * (OPTOK // 16):
                                  c * (OPTOK // 16) + op_tok[c] // 16],
                    num_idxs=op_tok[c],
                    num_idxs_reg=op_tok[c],
                    elem_size=EW,
                    single_packet=False,
                )
                gtiles.append(g)

            for t in range(Tc):
                mm = BLK[t]
                c, t_loc = t // (OPTOK // 128), t % (OPTOK // 128)
                g = gtiles[c]
                for x in range(3):
                    for q in range(nquad):
                        nc.tensor.matmul(
                            ps_cd[x][:, bass.ts(q, QW * D)],
                            lhsT=P_sb[:, bass.ts(t, 128)],
                            rhs=g[:, t_loc,
                                  (x * n_rows + QW * q) * D:
                                  (x * n_rows + QW * (q + 1)) * D],
                            start=(t == meta.first_chunk[mm]),
                            stop=(t == meta.last_chunk[mm]),
                        )
                if t == meta.last_chunk[mm]:
                    for x, dst in ((0, qc_cd), (1, kc_cd), (2, v_aug)):
                        nc.vector.tensor_scalar_mul(
                            dst[:, :, mm, 0:D], ps_cd[x][:, :],
                            w_sb[:, mm:mm + 1])

            ph1.__exit__(None, None, None)

            # ---- phase 2/3 PSUM pools
            ph2a = tc.tile_pool(name="psqk", bufs=2, space="PSUM")
            psqkp = ph2a.__enter__()
            ph2b = tc.tile_pool(name="psmall", bufs=4, space="PSUM")
            psmallp = ph2b.__enter__()

            # ---- phase 2+3 interleaved by row halves
            def phase2(r):
                qdc = rowp.tile([128, NBLK, 128], dt.bfloat16, tag="qdc")
                kdc = rowp.tile([128, NBLK, 128], dt.bfloat16, tag="kdc")
                for src, dst in ((qc_cd, qdc), (kc_cd, kdc)):
                    for mm in range(NBLK):
                        pst = psmallp.tile([128, 128], dt.bfloat16, tag="psmall")
                        nc.tensor.transpose(pst[:, :], src[:, r, mm, 0:D],
                                            ident_sb[:, :])
                        nc.vector.tensor_copy(dst[:, mm, :], pst[:, :])
                qdc_f = qdc[:, :, :].rearrange("p a b -> p (a b)")

                aT = rowp.tile([128, NBLK, 520], dt.bfloat16, tag="aT")
                for j in range(NBLK):
                    psA = psqkp.tile([128, 512], dt.float32, tag="psqk")
                    psB = psmallp.tile([128, 8], dt.float32, tag="psmall")
                    nc.tensor.matmul(psA[:, :], lhsT=kdc[:, j, :],
                                     rhs=qdc_f[:, 0:512], start=True, stop=True)
                    nc.tensor.matmul(psB[:, :], lhsT=kdc[:, j, :],
                                     rhs=qdc_f[:, 512:520], start=True, stop=True)
                    nc.scalar.activation(aT[:, j, 0:512], psA[:, :], Exp,
                                         bias=ln_sb[:, j:j + 1])
                    nc.scalar.activation(aT[:, j, 512:520], psB[:, :], Exp,
                                         bias=ln_sb[:, j:j + 1])

                for i in range(NBLK):
                    M = 128 if i < NBLK - 1 else (C - 128 * (NBLK - 1))
                    psv = psmallp.tile([M, D + 4], dt.float32, tag="psmall")
                    for j in range(NBLK):
                        nc.tensor.matmul(
                            psv[:, 0:D + 2],
                            lhsT=aT[:, j, 128 * i:128 * i + M],
                            rhs=v_aug[:, r, j, 0:D + 2],
                            start=(j == 0), stop=(j == NBLK - 1),
                        )
                    rv = smallp.tile([M, 1], dt.float32, tag="rv")
                    nc.vector.reciprocal(rv[:, :], psv[:, D:D + 1])
                    if M < 128:
                        nc.vector.memset(v_nrm[:, r, i, :], 0.0)
                    nc.vector.tensor_scalar_mul(v_nrm[0:M, r, i, :],
                                                psv[:, 0:D], rv[:, :])
                    nc.vector.tensor_mul(a0_all[0:M, i, r:r + 1],
                                         psv[:, D + 1:D + 2], rv[:, :])

            def phase3(half, rlo, nr, ext):
                hw_ = nr * D
                for c in range(n_ops):
                    ntok = op_tok[c]
                    osort = osortp.tile([128, OPTOK // 128, hw_], dt.float32,
                                        tag=f"osort{half}",
                                        name=f"osort{half}_{c}")
                    for t_loc in range(ntok // 128):
                        t = c * (OPTOK // 128) + t_loc
                        pso = psmallp.tile([128, hw_], dt.float32,
                                           tag="psmall", name=f"pso{half}_{t}")
                        nc.tensor.matmul(
                            pso[:, :],
                            lhsT=PT_sb[:, bass.ts(t, 128)],
                            rhs=v_nrm[:, rlo:rlo + nr, BLK[t], :],
                            start=True, stop=True)
                        if t_loc % 2:
                            nc.scalar.copy(osort[:, t_loc, :], pso[:, :])
                        else:
                            nc.vector.tensor_copy(osort[:, t_loc, :], pso[:, :])
                    nc.gpsimd.dma_scatter_add(
                        ext[:, :],
                        osort[:, 0:ntok // 128, :],
                        is_sb[:, c * (OPTOK // 16):
                              c * (OPTOK // 16) + ntok // 16],
                        ntok,
                        ntok,
                        hw_,
                        single_packet=False,
                    )

            for r in range(HR):
                phase2(r)
            phase3(0, 0, HR, outA_ext)
            for r in range(HR, n_rows):
                phase2(r)
            if n_rows > HR:
                phase3(1, HR, n_rows - HR, outB_ext)
            nc.sync.dma_start(a0_ext[:, :],
                              a0_all[:, :, :].rearrange("p a b -> p (a b)"))
            ph2b.__exit__(None, None, None)
            ph2a.__exit__(None, None, None)

    return nc


# ------------------------------------------------------------------- runner

_CACHE = {}


def _ensure_ntff_hook():
    import types
    if "antenv.axon_hooks" in sys.modules:
        return
    m = types.ModuleType("antenv.axon_hooks")
    m._hook = None
    m.set_axon_ntff_profile_hook = lambda h: setattr(m, "_hook", h)
    m.get_axon_ntff_profile_hook = lambda: m._hook
    sys.modules["antenv.axon_hooks"] = m
    try:
        import antenv
        antenv.axon_hooks = m
    except Exception:
        pass
    try:
        from trn_agent_boot.trn_boot import _ntff_profile_via_ctypes
        hook = _ntff_profile_via_ctypes("/opt/axon/libaxon_pjrt.so")
        if hook is not None:
            m._hook = hook
    except Exception:
        pass


def make_in_maps(queries, keys, values, meta, n_rows=ROWS):
    bf16 = _bf16()
    in_maps = []
    for i in range(NCORES):
        cm = meta.cores[i]
        # token-major pack: qkv8[n, x*n_rows + r, :] = X_x[i + 8r, n, :]
        qkv = np.empty((NSEQ, 3, n_rows, D), dtype=bf16)
        for x, src in enumerate((queries, keys, values)):
            rows = src[i::NCORES][:n_rows]          # [n_rows, NSEQ, D]
            qkv[:, x, :, :] = rows.transpose(1, 0, 2).astype(bf16)
        in_maps.append({
            "qkv": qkv.reshape(NSEQ, 3 * n_rows * D),
            "P": cm.P_sb,
            "PT": cm.PT_sb,
            "idxg": cm.idx_g,
            "idxs": cm.idx_s,
            "wcol": cm.wcol,
            "lncnt": cm.lncnt,
        })
    return in_maps


def run_cores(queries, keys, values, clusters, trace=False, n_rows=ROWS):
    _ensure_ntff_hook()
    from concourse.bass_utils import run_bass_kernel_spmd

    meta = build_meta(np.asarray(clusters))
    key = (meta.T_m, n_rows)
    if key not in _CACHE:
        nc = build_nc(meta, n_rows)
        nc.finalize()
        _CACHE[key] = nc
    nc = _CACHE[key]
    in_maps = make_in_maps(np.asarray(queries), np.asarray(keys),
                           np.asarray(values), meta, n_rows)
    res = run_bass_kernel_spmd(nc, in_maps, core_ids=list(range(NCORES)),
                               trace=trace)
    return res, meta


def unshard(res, n_rows=ROWS):
    out = np.empty((B, NSEQ, D), np.float32)
    a0 = np.empty((B, C), np.float32)
    HR = max(n_rows // 2, 1)
    for i in range(NCORES):
        r = res.results[i]
        oA = r["outA"][:NSEQ].reshape(NSEQ, HR, D)
        for rr in range(HR):
            out[i + NCORES * rr] = oA[:, rr, :]
        if n_rows > HR:
            oB = r["outB"][:NSEQ].reshape(NSEQ, n_rows - HR, D)
            for rr in range(HR, n_rows):
                out[i + NCORES * rr] = oB[:, rr - HR, :]
        a0T = r["a0T"].reshape(128, NBLK, n_rows)
        for rr in range(n_rows):
            a0[i + NCORES * rr] = a0T[:, :, rr].T.reshape(CPAD)[:C]
    return out, a0


def kernel(**inputs):
    queries = np.asarray(inputs["queries"], np.float32)
    keys = np.asarray(inputs["keys"], np.float32)
    values = np.asarray(inputs["values"], np.float32)
    clusters = np.asarray(inputs["clusters"], np.int32)
    res, _ = run_cores(queries, keys, values, clusters, trace=False)
    return unshard(res)
